# revision 8
# baseline (speedup 1.0000x reference)
"""ConceptGNN (2-layer GCN + word gather) on 8 trn2 NeuronCores via Bass/Tile.

Strategy (hardcoded for V=50000, D=300, G=256, E=1.6e6, B=S=64, 8 cores):
  - Nodes dst-sharded: core c owns rows [c*6250, (c+1)*6250).
  - Layer 1: every core redundantly computes H1' = (emb @ W1) * rsqrt(deg)
    (cheaper than all-gathering a 50MB tensor), then aggregates only its own
    dst shard: edges bucketed by (dst block of 128, src half) on host,
    gathered edge-major with gpsimd.dma_gather (int16 indices -> two source
    halves), segment-summed by one-hot matmuls accumulating in PSUM.
    Self-loop terms are appended as explicit u->u edges so the device
    program is identical across cores (pure SPMD).
  - Layer 2: only rows needed by word_ids are produced. Each core computes
    H2' = (x2_shard @ W2) * rsqrt(deg) for its local rows, partial-aggregates
    word-destination edges whose src lives in its shard, ReduceScatter over
    the 8 cores (word slots are grouped by owner core), and finalizes its
    owned word rows. Host reassembles the (B,S,G) output by pure indexing.
"""

import os
import numpy as np

V, D, G = 50000, 300, 256
M = 8                      # cores
VS = V // M                # 6250 rows per shard
NB1 = (VS + 127) // 128    # 49 dst blocks per core
VPAD = 128 * ((V + 127) // 128 + 1)  # 50176, multiple of 128
HLO = 32768                # int16-addressable row limit for dma_gather
SUP = 4                    # row tiles per supertile in H' build

_CACHE = {}

# filled by the last kernel() call when KERNEL_TRACE=1
last_exec_time_ns = None
last_results = None


def _round_up(x, m):
    return (x + m - 1) // m * m


def _idx_tile16(arr_i16):
    """dma_gather index layout: idx k -> [k%16, k//16], replicated to 128 parts."""
    n = arr_i16.shape[0]
    t = arr_i16.reshape(n // 16, 16).T  # [16, n/16]
    return np.tile(t, (8, 1))           # [128, n/16]


def _col_tile(arr, T):
    """edge-major column layout: edge k -> [k%128, k//128]; arr len T*128."""
    return arr.reshape(T, 128).T        # [128, T]


def _plan(emb, W1, b1, W2, b2, edge_index, word_ids):
    """Host preprocessing: sharding, edge bucketing, padded index arrays."""
    src = np.asarray(edge_index[0], dtype=np.int64).astype(np.int32)
    dst = np.asarray(edge_index[1], dtype=np.int64).astype(np.int32)
    E = src.shape[0]

    deg = (1.0 + np.bincount(dst, minlength=V)).astype(np.float32)

    # append self edges (u -> u): covers the self-loop term of GCNConv
    src_all = np.concatenate([src, np.arange(V, dtype=np.int32)])
    dst_all = np.concatenate([dst, np.arange(V, dtype=np.int32)])

    # ---- layer-1 buckets: per (core, dst-block of 128, src-half) ----
    core_of = dst_all // VS
    dloc = dst_all % VS
    blk = dloc // 128
    half = (src_all >= HLO).astype(np.int32)

    per_core = []
    n_lo_max, n_hi_max = 1, 1
    for c in range(M):
        sel = np.where(core_of == c)[0]
        order = np.lexsort((half[sel], blk[sel]))
        sel = sel[order]
        s_c, b_c, h_c = src_all[sel], blk[sel], half[sel]
        dl_c = (dloc[sel] - b_c * 128).astype(np.float32)
        # boundaries per (block, half)
        buckets = []
        for b in range(NB1):
            ib = np.where(b_c == b)[0]
            ilo = ib[h_c[ib] == 0]
            ihi = ib[h_c[ib] == 1]
            buckets.append((s_c[ilo], dl_c[ilo], s_c[ihi] - HLO, dl_c[ihi]))
            n_lo_max = max(n_lo_max, len(ilo))
            n_hi_max = max(n_hi_max, len(ihi))
        per_core.append(buckets)

    T_LO = _round_up(n_lo_max, 128) // 128
    T_HI = _round_up(n_hi_max, 128) // 128
    T1 = T_LO + T_HI

    IDX1 = np.zeros((M, NB1, 128, T1 * 8), np.int16)
    DSTL1 = np.full((M, NB1, 128, T1), -1.0, np.float32)
    for c in range(M):
        for b in range(NB1):
            slo, dlo, shi, dhi = per_core[c][b]
            a = np.zeros(T_LO * 128, np.int16)
            a[: len(slo)] = slo.astype(np.int16)
            IDX1[c, b, :, : T_LO * 8] = _idx_tile16(a)
            d = np.full(T_LO * 128, -1.0, np.float32)
            d[: len(dlo)] = dlo
            DSTL1[c, b, :, :T_LO] = _col_tile(d, T_LO)
            a = np.zeros(T_HI * 128, np.int16)
            a[: len(shi)] = shi.astype(np.int16)
            IDX1[c, b, :, T_LO * 8:] = _idx_tile16(a)
            d = np.full(T_HI * 128, -1.0, np.float32)
            d[: len(dhi)] = dhi
            DSTL1[c, b, :, T_LO:] = _col_tile(d, T_HI)

    # ---- layer-2: word nodes, owner-grouped slots ----
    words = np.unique(np.asarray(word_ids, dtype=np.int64).astype(np.int32))
    owner = words // VS
    wlists = [words[owner == c] for c in range(M)]
    MW = max(max(len(w) for w in wlists), 1)
    W_PAD = _round_up(MW, 128)
    NWB = M * W_PAD // 128

    slot_map = np.full(V, -1, np.int64)
    for c in range(M):
        slot_map[wlists[c]] = c * W_PAD + np.arange(len(wlists[c]))

    wm = slot_map[dst] >= 0
    wsrc = np.concatenate([src[wm], words])
    wslot = np.concatenate([slot_map[dst[wm]], slot_map[words]]).astype(np.int32)

    wcore = wsrc // VS
    wblk = wslot // 128
    wloc = (wsrc % VS).astype(np.int32)
    wdl = (wslot % 128).astype(np.float32)

    n_w_max = 1
    wbuckets = []
    for c in range(M):
        sel = np.where(wcore == c)[0]
        order = np.argsort(wblk[sel], kind="stable")
        sel = sel[order]
        bl = []
        for b in range(NWB):
            ib = sel[wblk[sel] == b]
            bl.append((wloc[ib], wdl[ib]))
            n_w_max = max(n_w_max, len(ib))
        wbuckets.append(bl)
    T_W = _round_up(n_w_max, 128) // 128

    WIDX = np.zeros((M, NWB, 128, T_W * 8), np.int16)
    WDSTL = np.full((M, NWB, 128, T_W), -1.0, np.float32)
    for c in range(M):
        for b in range(NWB):
            ls, dl = wbuckets[c][b]
            a = np.zeros(T_W * 128, np.int16)
            a[: len(ls)] = ls.astype(np.int16)
            WIDX[c, b] = _idx_tile16(a)
            d = np.full(T_W * 128, -1.0, np.float32)
            d[: len(dl)] = dl
            WDSTL[c, b] = _col_tile(d, T_W)

    # ---- degree tensors ----
    degp = np.ones(VPAD, np.float32)
    degp[:V] = deg
    deg_glob = degp.reshape(VPAD // 128, 128).T.copy()  # [128, 392]

    deg_blk = np.ones((M, 128, NB1), np.float32)
    deg_w = np.ones((M, 128, W_PAD // 128), np.float32)
    for c in range(M):
        d = np.ones(NB1 * 128, np.float32)
        d[:VS] = deg[c * VS:(c + 1) * VS]
        deg_blk[c] = d.reshape(NB1, 128).T
        d = np.ones(W_PAD, np.float32)
        d[: len(wlists[c])] = deg[wlists[c]]
        deg_w[c] = d.reshape(W_PAD // 128, 128).T

    # ---- dense inputs ----
    embT = np.zeros((D, VPAD), np.float32)
    embT[:, :V] = np.asarray(emb, np.float32).T
    W1f = np.asarray(W1, np.float32)
    W2f = np.asarray(W2, np.float32)
    b1rep = np.broadcast_to(np.asarray(b1, np.float32), (128, G)).copy()
    b2rep = np.broadcast_to(np.asarray(b2, np.float32), (128, G)).copy()
    iota = np.broadcast_to(np.arange(128, dtype=np.float32), (128, 128)).copy()

    cfg = (T_LO, T_HI, W_PAD, NWB, T_W)
    in_maps = []
    for c in range(M):
        in_maps.append({
            "embT": embT, "W1": W1f, "W2": W2f,
            "b1rep": b1rep, "b2rep": b2rep, "iota": iota,
            "deg_glob": deg_glob, "deg_blk": deg_blk[c], "deg_w": deg_w[c],
            "idx1": IDX1[c], "dstl1": DSTL1[c],
            "widx": WIDX[c], "wdstl": WDSTL[c],
        })
    return cfg, in_maps, slot_map


def _build(cfg):
    from concourse import mybir, bacc
    import concourse.tile as tile

    T_LO, T_HI, W_PAD, NWB, T_W = cfg
    T1 = T_LO + T_HI
    dt = mybir.dt
    AF = mybir.ActivationFunctionType
    OP = mybir.AluOpType

    nc = bacc.Bacc("TRN2", target_bir_lowering=False, debug=False, num_devices=M)

    def din(name, shape, d=dt.float32):
        return nc.dram_tensor(name, shape, d, kind="ExternalInput").ap()

    embT = din("embT", [D, VPAD])
    W1 = din("W1", [D, G])
    W2 = din("W2", [G, G])
    b1rep = din("b1rep", [128, G])
    b2rep = din("b2rep", [128, G])
    iota_in = din("iota", [128, 128])
    deg_glob = din("deg_glob", [128, VPAD // 128])
    deg_blk = din("deg_blk", [128, NB1])
    deg_w = din("deg_w", [128, W_PAD // 128])
    idx1 = din("idx1", [NB1, 128, T1 * 8], dt.int16)
    dstl1 = din("dstl1", [NB1, 128, T1])
    widx = din("widx", [NWB, 128, T_W * 8], dt.int16)
    wdstl = din("wdstl", [NWB, 128, T_W])

    out_words = nc.dram_tensor("out_words", [W_PAD, G], dt.float32,
                               kind="ExternalOutput").ap()

    HP = nc.dram_tensor("HP", [VPAD, G], dt.float32).ap()        # H1'
    X2T = nc.dram_tensor("X2T", [G, NB1 * 128], dt.float32).ap()  # x2 transposed
    H2P = nc.dram_tensor("H2P", [NB1 * 128, G], dt.float32).ap()  # H2' local

    NGT = VPAD // 128          # 392 row tiles
    NSUP = NGT // SUP          # 98 supertiles

    with tile.TileContext(nc) as tc:
        with tc.tile_pool(name="const", bufs=1) as cpool, \
             tc.tile_pool(name="emb", bufs=3) as epool, \
             tc.tile_pool(name="hp", bufs=3) as hpool, \
             tc.tile_pool(name="mm1", bufs=2, space="PSUM") as mm1psum, \
             tc.tile_pool(name="gath", bufs=2) as gpool, \
             tc.tile_pool(name="oh", bufs=2) as ohpool, \
             tc.tile_pool(name="meta", bufs=3) as mpool, \
             tc.tile_pool(name="agg", bufs=2, space="PSUM") as aggpsum, \
             tc.tile_pool(name="fin", bufs=3) as fpool, \
             tc.tile_pool(name="trp", bufs=2, space="PSUM") as trpsum, \
             tc.tile_pool(name="dram", bufs=1, space="DRAM") as dram:

            # ---------- constants ----------
            iota_t = cpool.tile([128, 128], dt.float32)
            nc.sync.dma_start(out=iota_t[:], in_=iota_in[:])
            ident = cpool.tile([128, 128], dt.float32)
            from concourse.masks import make_identity
            make_identity(nc, ident[:])
            KT = [(0, 128), (128, 128), (256, D - 256)]
            w1_t = cpool.tile([128, 3, G], dt.float32)
            for ki, (k0, kk) in enumerate(KT):
                nc.sync.dma_start(out=w1_t[:kk, ki, :], in_=W1[k0:k0 + kk, :])
            w2_t = cpool.tile([128, 2, G], dt.float32)
            for j in range(2):
                nc.sync.dma_start(out=w2_t[:, j, :], in_=W2[j * 128:(j + 1) * 128, :])
            b1_t = cpool.tile([128, G], dt.float32)
            nc.sync.dma_start(out=b1_t[:], in_=b1rep[:])
            b2_t = cpool.tile([128, G], dt.float32)
            nc.sync.dma_start(out=b2_t[:], in_=b2rep[:])

            invg = cpool.tile([128, VPAD // 128], dt.float32)
            nc.sync.dma_start(out=invg[:], in_=deg_glob[:])
            nc.scalar.activation(invg[:], invg[:], AF.Sqrt)
            nc.vector.reciprocal(invg[:], invg[:])
            invb = cpool.tile([128, NB1], dt.float32)
            nc.sync.dma_start(out=invb[:], in_=deg_blk[:])
            nc.scalar.activation(invb[:], invb[:], AF.Sqrt)
            nc.vector.reciprocal(invb[:], invb[:])
            invw = cpool.tile([128, W_PAD // 128], dt.float32)
            nc.sync.dma_start(out=invw[:], in_=deg_w[:])
            nc.scalar.activation(invw[:], invw[:], AF.Sqrt)
            nc.vector.reciprocal(invw[:], invw[:])

            # ---------- phase 1: H1' = (emb @ W1) * invg, replicated ----------
            for s in range(NSUP):
                et = epool.tile([128, 3, SUP * 128], dt.float32, tag="embt")
                for ki, (k0, kk) in enumerate(KT):
                    nc.sync.dma_start(
                        out=et[:kk, ki, :],
                        in_=embT[k0:k0 + kk, s * SUP * 128:(s + 1) * SUP * 128])
                hpt = hpool.tile([128, SUP, G], dt.float32, tag="hp")
                for r in range(SUP):
                    ps = mm1psum.tile([128, G], dt.float32)
                    for ki, (k0, kk) in enumerate(KT):
                        nc.tensor.matmul(
                            out=ps[:],
                            lhsT=et[:kk, ki, r * 128:(r + 1) * 128],
                            rhs=w1_t[:kk, ki, :],
                            start=(ki == 0), stop=(ki == 2))
                    col = s * SUP + r
                    nc.vector.tensor_scalar(
                        out=hpt[:, r, :], in0=ps[:],
                        scalar1=invg[:, col:col + 1], scalar2=None, op0=OP.mult)
                    nc.sync.dma_start(
                        out=HP[col * 128:(col + 1) * 128, :], in_=hpt[:, r, :])

            # ---------- phase 2: layer-1 aggregation over own dst shard ----------
            for b in range(NB1):
                it = mpool.tile([128, T1 * 8], dt.int16, tag="idx")
                nc.sync.dma_start(out=it[:], in_=idx1[b])
                dt_t = mpool.tile([128, T1], dt.float32, tag="dstl")
                nc.sync.dma_start(out=dt_t[:], in_=dstl1[b])

                # gathers chunked to <=512 idxs (SWDGE descriptor ring is 1024)
                CH = 4
                gb = gpool.tile([128, T1, G], dt.float32, tag="gb")
                for t0 in range(0, T_LO, CH):
                    n = min(CH, T_LO - t0)
                    nc.gpsimd.dma_gather(
                        gb[:, t0:t0 + n, :], HP[0:HLO, :],
                        it[:, t0 * 8:(t0 + n) * 8],
                        num_idxs=n * 128, num_idxs_reg=n * 128, elem_size=G)
                for t0 in range(0, T_HI, CH):
                    n = min(CH, T_HI - t0)
                    nc.gpsimd.dma_gather(
                        gb[:, T_LO + t0:T_LO + t0 + n, :], HP[HLO:VPAD, :],
                        it[:, (T_LO + t0) * 8:(T_LO + t0 + n) * 8],
                        num_idxs=n * 128, num_idxs_reg=n * 128, elem_size=G)

                ps = aggpsum.tile([128, G], dt.float32)
                for t0 in range(0, T1, CH):
                    n = min(CH, T1 - t0)
                    oh = ohpool.tile([128, CH, 128], dt.float32, tag="oh")
                    nc.vector.tensor_tensor(
                        out=oh[:, :n, :],
                        in0=dt_t[:, t0:t0 + n, None].to_broadcast([128, n, 128]),
                        in1=iota_t[:, None, :].to_broadcast([128, n, 128]),
                        op=OP.is_equal)
                    for t in range(n):
                        nc.tensor.matmul(out=ps[:], lhsT=oh[:, t, :],
                                         rhs=gb[:, t0 + t, :],
                                         start=(t0 + t == 0),
                                         stop=(t0 + t == T1 - 1))

                x2 = fpool.tile([128, G], dt.float32, tag="x2")
                nc.vector.tensor_scalar(
                    out=x2[:], in0=ps[:],
                    scalar1=invb[:, b:b + 1], scalar2=None, op0=OP.mult)
                nc.vector.tensor_add(out=x2[:], in0=x2[:], in1=b1_t[:])
                nc.scalar.activation(x2[:], x2[:], AF.Relu)

                # transpose x2 block -> X2T columns
                for j in range(2):
                    tp = trpsum.tile([128, 128], dt.float32)
                    nc.tensor.transpose(
                        out=tp[:], in_=x2[:, j * 128:(j + 1) * 128],
                        identity=ident[:])
                    x2tc = fpool.tile([128, 128], dt.float32, tag="x2t")
                    nc.vector.tensor_copy(out=x2tc[:], in_=tp[:])
                    nc.sync.dma_start(
                        out=X2T[j * 128:(j + 1) * 128, b * 128:(b + 1) * 128],
                        in_=x2tc[:])

            # ---------- phase 4: H2' = (x2 @ W2) * inv, local rows ----------
            for r in range(NB1):
                lt = epool.tile([128, 2, 128], dt.float32, tag="x2tl")
                for j in range(2):
                    nc.sync.dma_start(
                        out=lt[:, j, :],
                        in_=X2T[j * 128:(j + 1) * 128, r * 128:(r + 1) * 128])
                ps = mm1psum.tile([128, G], dt.float32)
                for j in range(2):
                    nc.tensor.matmul(out=ps[:], lhsT=lt[:, j, :],
                                     rhs=w2_t[:, j, :],
                                     start=(j == 0), stop=(j == 1))
                h2 = hpool.tile([128, G], dt.float32, tag="h2")
                nc.vector.tensor_scalar(
                    out=h2[:], in0=ps[:],
                    scalar1=invb[:, r:r + 1], scalar2=None, op0=OP.mult)
                nc.sync.dma_start(out=H2P[r * 128:(r + 1) * 128, :], in_=h2[:])

            # ---------- phase 5: word partial aggregation ----------
            wpart = dram.tile([M * W_PAD, G], dt.float32)
            for b in range(NWB):
                it = mpool.tile([128, T_W * 8], dt.int16, tag="widx")
                nc.sync.dma_start(out=it[:], in_=widx[b])
                dt_t = mpool.tile([128, T_W], dt.float32, tag="wdstl")
                nc.sync.dma_start(out=dt_t[:], in_=wdstl[b])

                CH = 4
                gb = gpool.tile([128, T_W, G], dt.float32, tag="wgb")
                for t0 in range(0, T_W, CH):
                    n = min(CH, T_W - t0)
                    nc.gpsimd.dma_gather(
                        gb[:, t0:t0 + n, :], H2P[:], it[:, t0 * 8:(t0 + n) * 8],
                        num_idxs=n * 128, num_idxs_reg=n * 128, elem_size=G)

                ps = aggpsum.tile([128, G], dt.float32, tag="wps")
                for t0 in range(0, T_W, CH):
                    n = min(CH, T_W - t0)
                    oh = ohpool.tile([128, CH, 128], dt.float32, tag="woh")
                    nc.vector.tensor_tensor(
                        out=oh[:, :n, :],
                        in0=dt_t[:, t0:t0 + n, None].to_broadcast([128, n, 128]),
                        in1=iota_t[:, None, :].to_broadcast([128, n, 128]),
                        op=OP.is_equal)
                    for t in range(n):
                        nc.tensor.matmul(out=ps[:], lhsT=oh[:, t, :],
                                         rhs=gb[:, t0 + t, :],
                                         start=(t0 + t == 0),
                                         stop=(t0 + t == T_W - 1))
                pt = fpool.tile([128, G], dt.float32, tag="wpt")
                nc.vector.tensor_copy(out=pt[:], in_=ps[:])
                nc.sync.dma_start(out=wpart[b * 128:(b + 1) * 128, :], in_=pt[:])

            # ---------- phase 6: ReduceScatter partials ----------
            wred = dram.tile([W_PAD, G], dt.float32)
            nc.gpsimd.collective_compute(
                "ReduceScatter", OP.add,
                replica_groups=[list(range(M))],
                ins=[wpart.opt()], outs=[wred.opt()])

            # ---------- phase 7: finalize owned word rows ----------
            for t in range(W_PAD // 128):
                wt = fpool.tile([128, G], dt.float32, tag="wfin")
                nc.sync.dma_start(out=wt[:], in_=wred[t * 128:(t + 1) * 128, :])
                nc.vector.tensor_scalar(
                    out=wt[:], in0=wt[:],
                    scalar1=invw[:, t:t + 1], scalar2=None, op0=OP.mult)
                nc.vector.tensor_add(out=wt[:], in0=wt[:], in1=b2_t[:])
                nc.sync.dma_start(out=out_words[t * 128:(t + 1) * 128, :],
                                  in_=wt[:])

    nc.compile()
    return nc


def _install_ntff_hook():
    import sys, types
    import antenv
    from concourse import bass_utils
    if "antenv.axon_hooks" in sys.modules:
        return
    from trn_agent_boot.trn_boot import _ntff_profile_via_ctypes
    hooks = types.ModuleType("antenv.axon_hooks")
    _h = [None]
    hooks.set_axon_ntff_profile_hook = lambda h: _h.__setitem__(0, h)
    hooks.get_axon_ntff_profile_hook = lambda: _h[0]
    sys.modules["antenv.axon_hooks"] = hooks
    antenv.axon_hooks = hooks
    hooks.set_axon_ntff_profile_hook(
        _ntff_profile_via_ctypes("/opt/axon/libaxon_pjrt.so"))
    bass_utils.upload_artifacts = lambda tmpdir: f"local:{tmpdir}"


def kernel(**inputs):
    global last_exec_time_ns, last_results
    from concourse.bass_utils import run_bass_kernel_spmd

    cfg, in_maps, slot_map = _plan(
        inputs["emb"], inputs["W1"], inputs["b1"], inputs["W2"], inputs["b2"],
        inputs["edge_index"], inputs["word_ids"])

    if cfg not in _CACHE:
        _CACHE[cfg] = _build(cfg)
    nc = _CACHE[cfg]

    trace = os.environ.get("KERNEL_TRACE", "0") == "1"
    if trace:
        _install_ntff_hook()
    res = run_bass_kernel_spmd(nc, in_maps, core_ids=list(range(M)), trace=trace)
    last_exec_time_ns = res.exec_time_ns
    last_results = res

    W_PAD = cfg[2]
    word_ids = np.asarray(inputs["word_ids"], dtype=np.int64)
    all_rows = np.concatenate([res.results[c]["out_words"] for c in range(M)])
    out = all_rows[slot_map[word_ids.ravel()]].reshape(*word_ids.shape, G)
    return out.astype(np.float32)


# revision 9
# speedup vs baseline: 1.0017x; 1.0017x over previous
"""ConceptGNN (2-layer GCN + word gather) on 8 trn2 NeuronCores via Bass/Tile.

Strategy (hardcoded for V=50000, D=300, G=256, E=1.6e6, B=S=64, 8 cores):
  - Nodes dst-sharded: core c owns rows [c*6250, (c+1)*6250).
  - Layer 1: every core redundantly computes H1' = (emb @ W1) * rsqrt(deg)
    (cheaper than all-gathering a 50MB tensor), then aggregates only its own
    dst shard: edges bucketed by (dst block of 128, src half) on host,
    gathered edge-major with gpsimd.dma_gather (int16 indices -> two source
    halves), segment-summed by one-hot matmuls accumulating in PSUM.
    Self-loop terms are appended as explicit u->u edges so the device
    program is identical across cores (pure SPMD).
  - Layer 2: only rows needed by word_ids are produced. Each core computes
    H2' = (x2_shard @ W2) * rsqrt(deg) for its local rows, partial-aggregates
    word-destination edges whose src lives in its shard, ReduceScatter over
    the 8 cores (word slots are grouped by owner core), and finalizes its
    owned word rows. Host reassembles the (B,S,G) output by pure indexing.
"""

import os
import numpy as np

V, D, G = 50000, 300, 256
M = 8                      # cores
VS = V // M                # 6250 rows per shard
NB1 = (VS + 127) // 128    # 49 dst blocks per core
VPAD = 128 * ((V + 127) // 128 + 1)  # 50176, multiple of 128
HLO = 32768                # int16-addressable row limit for dma_gather
SUP = 4                    # row tiles per supertile in H' build

_CACHE = {}

# filled by the last kernel() call when KERNEL_TRACE=1
last_exec_time_ns = None
last_results = None


def _round_up(x, m):
    return (x + m - 1) // m * m


def _idx_tile16(arr_i16):
    """dma_gather index layout: idx k -> [k%16, k//16], replicated to 128 parts."""
    n = arr_i16.shape[0]
    t = arr_i16.reshape(n // 16, 16).T  # [16, n/16]
    return np.tile(t, (8, 1))           # [128, n/16]


def _col_tile(arr, T):
    """edge-major column layout: edge k -> [k%128, k//128]; arr len T*128."""
    return arr.reshape(T, 128).T        # [128, T]


def _plan(emb, W1, b1, W2, b2, edge_index, word_ids):
    """Host preprocessing: sharding, edge bucketing, padded index arrays."""
    src = np.asarray(edge_index[0], dtype=np.int64).astype(np.int32)
    dst = np.asarray(edge_index[1], dtype=np.int64).astype(np.int32)
    E = src.shape[0]

    deg = (1.0 + np.bincount(dst, minlength=V)).astype(np.float32)

    # append self edges (u -> u): covers the self-loop term of GCNConv
    src_all = np.concatenate([src, np.arange(V, dtype=np.int32)])
    dst_all = np.concatenate([dst, np.arange(V, dtype=np.int32)])

    # ---- layer-1 buckets: per (core, dst-block of 128, src-half) ----
    core_of = dst_all // VS
    dloc = dst_all % VS
    blk = dloc // 128
    half = (src_all >= HLO).astype(np.int32)

    per_core = []
    n_lo_max, n_hi_max = 1, 1
    for c in range(M):
        sel = np.where(core_of == c)[0]
        order = np.lexsort((half[sel], blk[sel]))
        sel = sel[order]
        s_c, b_c, h_c = src_all[sel], blk[sel], half[sel]
        dl_c = (dloc[sel] - b_c * 128).astype(np.float32)
        # boundaries per (block, half)
        buckets = []
        for b in range(NB1):
            ib = np.where(b_c == b)[0]
            ilo = ib[h_c[ib] == 0]
            ihi = ib[h_c[ib] == 1]
            buckets.append((s_c[ilo], dl_c[ilo], s_c[ihi] - HLO, dl_c[ihi]))
            n_lo_max = max(n_lo_max, len(ilo))
            n_hi_max = max(n_hi_max, len(ihi))
        per_core.append(buckets)

    T_LO = _round_up(n_lo_max, 128) // 128
    T_HI = _round_up(n_hi_max, 128) // 128
    T1 = T_LO + T_HI

    IDX1 = np.zeros((M, NB1, 128, T1 * 8), np.int16)
    DSTL1 = np.full((M, NB1, 128, T1), -1.0, np.float32)
    for c in range(M):
        for b in range(NB1):
            slo, dlo, shi, dhi = per_core[c][b]
            a = np.zeros(T_LO * 128, np.int16)
            a[: len(slo)] = slo.astype(np.int16)
            IDX1[c, b, :, : T_LO * 8] = _idx_tile16(a)
            d = np.full(T_LO * 128, -1.0, np.float32)
            d[: len(dlo)] = dlo
            DSTL1[c, b, :, :T_LO] = _col_tile(d, T_LO)
            a = np.zeros(T_HI * 128, np.int16)
            a[: len(shi)] = shi.astype(np.int16)
            IDX1[c, b, :, T_LO * 8:] = _idx_tile16(a)
            d = np.full(T_HI * 128, -1.0, np.float32)
            d[: len(dhi)] = dhi
            DSTL1[c, b, :, T_LO:] = _col_tile(d, T_HI)

    # ---- layer-2: word nodes, owner-grouped slots ----
    words = np.unique(np.asarray(word_ids, dtype=np.int64).astype(np.int32))
    owner = words // VS
    wlists = [words[owner == c] for c in range(M)]
    MW = max(max(len(w) for w in wlists), 1)
    W_PAD = _round_up(MW, 128)
    NWB = M * W_PAD // 128

    slot_map = np.full(V, -1, np.int64)
    for c in range(M):
        slot_map[wlists[c]] = c * W_PAD + np.arange(len(wlists[c]))

    wm = slot_map[dst] >= 0
    wsrc = np.concatenate([src[wm], words])
    wslot = np.concatenate([slot_map[dst[wm]], slot_map[words]]).astype(np.int32)

    wcore = wsrc // VS
    wblk = wslot // 128
    wloc = (wsrc % VS).astype(np.int32)
    wdl = (wslot % 128).astype(np.float32)

    n_w_max = 1
    wbuckets = []
    for c in range(M):
        sel = np.where(wcore == c)[0]
        order = np.argsort(wblk[sel], kind="stable")
        sel = sel[order]
        bl = []
        for b in range(NWB):
            ib = sel[wblk[sel] == b]
            bl.append((wloc[ib], wdl[ib]))
            n_w_max = max(n_w_max, len(ib))
        wbuckets.append(bl)
    T_W = _round_up(n_w_max, 128) // 128

    WIDX = np.zeros((M, NWB, 128, T_W * 8), np.int16)
    WDSTL = np.full((M, NWB, 128, T_W), -1.0, np.float32)
    for c in range(M):
        for b in range(NWB):
            ls, dl = wbuckets[c][b]
            a = np.zeros(T_W * 128, np.int16)
            a[: len(ls)] = ls.astype(np.int16)
            WIDX[c, b] = _idx_tile16(a)
            d = np.full(T_W * 128, -1.0, np.float32)
            d[: len(dl)] = dl
            WDSTL[c, b] = _col_tile(d, T_W)

    # ---- degree tensors ----
    degp = np.ones(VPAD, np.float32)
    degp[:V] = deg
    deg_glob = degp.reshape(VPAD // 128, 128).T.copy()  # [128, 392]

    deg_blk = np.ones((M, 128, NB1), np.float32)
    deg_w = np.ones((M, 128, W_PAD // 128), np.float32)
    for c in range(M):
        d = np.ones(NB1 * 128, np.float32)
        d[:VS] = deg[c * VS:(c + 1) * VS]
        deg_blk[c] = d.reshape(NB1, 128).T
        d = np.ones(W_PAD, np.float32)
        d[: len(wlists[c])] = deg[wlists[c]]
        deg_w[c] = d.reshape(W_PAD // 128, 128).T

    # ---- dense inputs ----
    embT = np.zeros((D, VPAD), np.float32)
    embT[:, :V] = np.asarray(emb, np.float32).T
    W1f = np.asarray(W1, np.float32)
    W2f = np.asarray(W2, np.float32)
    b1rep = np.broadcast_to(np.asarray(b1, np.float32), (128, G)).copy()
    b2rep = np.broadcast_to(np.asarray(b2, np.float32), (128, G)).copy()
    iota = np.broadcast_to(np.arange(128, dtype=np.float32), (128, 128)).copy()

    cfg = (T_LO, T_HI, W_PAD, NWB, T_W)
    in_maps = []
    for c in range(M):
        in_maps.append({
            "embT": embT, "W1": W1f, "W2": W2f,
            "b1rep": b1rep, "b2rep": b2rep, "iota": iota,
            "deg_glob": deg_glob, "deg_blk": deg_blk[c], "deg_w": deg_w[c],
            "idx1": IDX1[c], "dstl1": DSTL1[c],
            "widx": WIDX[c], "wdstl": WDSTL[c],
        })
    return cfg, in_maps, slot_map


def _build(cfg):
    from concourse import mybir, bacc
    import concourse.tile as tile

    T_LO, T_HI, W_PAD, NWB, T_W = cfg
    T1 = T_LO + T_HI
    dt = mybir.dt
    AF = mybir.ActivationFunctionType
    OP = mybir.AluOpType

    nc = bacc.Bacc("TRN2", target_bir_lowering=False, debug=False, num_devices=M)

    def din(name, shape, d=dt.float32):
        return nc.dram_tensor(name, shape, d, kind="ExternalInput").ap()

    embT = din("embT", [D, VPAD])
    W1 = din("W1", [D, G])
    W2 = din("W2", [G, G])
    b1rep = din("b1rep", [128, G])
    b2rep = din("b2rep", [128, G])
    iota_in = din("iota", [128, 128])
    deg_glob = din("deg_glob", [128, VPAD // 128])
    deg_blk = din("deg_blk", [128, NB1])
    deg_w = din("deg_w", [128, W_PAD // 128])
    idx1 = din("idx1", [NB1, 128, T1 * 8], dt.int16)
    dstl1 = din("dstl1", [NB1, 128, T1])
    widx = din("widx", [NWB, 128, T_W * 8], dt.int16)
    wdstl = din("wdstl", [NWB, 128, T_W])

    out_words = nc.dram_tensor("out_words", [W_PAD, G], dt.float32,
                               kind="ExternalOutput").ap()

    HP = nc.dram_tensor("HP", [VPAD, G], dt.float32).ap()        # H1'
    X2T = nc.dram_tensor("X2T", [G, NB1 * 128], dt.float32).ap()  # x2 transposed
    H2P = nc.dram_tensor("H2P", [NB1 * 128, G], dt.float32).ap()  # H2' local

    NGT = VPAD // 128          # 392 row tiles
    NSUP = NGT // SUP          # 98 supertiles

    with tile.TileContext(nc) as tc:
        with tc.tile_pool(name="const", bufs=1) as cpool, \
             tc.tile_pool(name="emb", bufs=3) as epool, \
             tc.tile_pool(name="hp", bufs=3) as hpool, \
             tc.tile_pool(name="mm1", bufs=2, space="PSUM") as mm1psum, \
             tc.tile_pool(name="gath", bufs=2) as gpool, \
             tc.tile_pool(name="oh", bufs=2) as ohpool, \
             tc.tile_pool(name="meta", bufs=3) as mpool, \
             tc.tile_pool(name="agg", bufs=2, space="PSUM") as aggpsum, \
             tc.tile_pool(name="fin", bufs=3) as fpool, \
             tc.tile_pool(name="trp", bufs=2, space="PSUM") as trpsum, \
             tc.tile_pool(name="dram", bufs=1, space="DRAM") as dram:

            # ---------- constants ----------
            iota_t = cpool.tile([128, 128], dt.float32)
            nc.sync.dma_start(out=iota_t[:], in_=iota_in[:])
            ident = cpool.tile([128, 128], dt.float32)
            from concourse.masks import make_identity
            make_identity(nc, ident[:])
            KT = [(0, 128), (128, 128), (256, D - 256)]
            w1_t = cpool.tile([128, 3, G], dt.float32)
            for ki, (k0, kk) in enumerate(KT):
                nc.sync.dma_start(out=w1_t[:kk, ki, :], in_=W1[k0:k0 + kk, :])
            w2_t = cpool.tile([128, 2, G], dt.float32)
            for j in range(2):
                nc.sync.dma_start(out=w2_t[:, j, :], in_=W2[j * 128:(j + 1) * 128, :])
            b1_t = cpool.tile([128, G], dt.float32)
            nc.sync.dma_start(out=b1_t[:], in_=b1rep[:])
            b2_t = cpool.tile([128, G], dt.float32)
            nc.sync.dma_start(out=b2_t[:], in_=b2rep[:])

            invg = cpool.tile([128, VPAD // 128], dt.float32)
            nc.sync.dma_start(out=invg[:], in_=deg_glob[:])
            nc.scalar.activation(invg[:], invg[:], AF.Sqrt)
            nc.vector.reciprocal(invg[:], invg[:])
            invb = cpool.tile([128, NB1], dt.float32)
            nc.sync.dma_start(out=invb[:], in_=deg_blk[:])
            nc.scalar.activation(invb[:], invb[:], AF.Sqrt)
            nc.vector.reciprocal(invb[:], invb[:])
            invw = cpool.tile([128, W_PAD // 128], dt.float32)
            nc.sync.dma_start(out=invw[:], in_=deg_w[:])
            nc.scalar.activation(invw[:], invw[:], AF.Sqrt)
            nc.vector.reciprocal(invw[:], invw[:])

            # ---------- phase 1: H1' = (emb @ W1) * invg, replicated ----------
            ctx_p1 = nc.named_scope("p1_hprime"); ctx_p1.__enter__()
            for s in range(NSUP):
                et = epool.tile([128, 3, SUP * 128], dt.float32, tag="embt")
                for ki, (k0, kk) in enumerate(KT):
                    nc.sync.dma_start(
                        out=et[:kk, ki, :],
                        in_=embT[k0:k0 + kk, s * SUP * 128:(s + 1) * SUP * 128])
                hpt = hpool.tile([128, SUP, G], dt.float32, tag="hp")
                for r in range(SUP):
                    ps = mm1psum.tile([128, G], dt.float32)
                    for ki, (k0, kk) in enumerate(KT):
                        nc.tensor.matmul(
                            out=ps[:],
                            lhsT=et[:kk, ki, r * 128:(r + 1) * 128],
                            rhs=w1_t[:kk, ki, :],
                            start=(ki == 0), stop=(ki == 2))
                    col = s * SUP + r
                    nc.vector.tensor_scalar(
                        out=hpt[:, r, :], in0=ps[:],
                        scalar1=invg[:, col:col + 1], scalar2=None, op0=OP.mult)
                    nc.sync.dma_start(
                        out=HP[col * 128:(col + 1) * 128, :], in_=hpt[:, r, :])

            ctx_p1.__exit__(None, None, None)
            # ---------- phase 2: layer-1 aggregation over own dst shard ----------
            ctx_p2 = nc.named_scope("p2_agg"); ctx_p2.__enter__()
            for b in range(NB1):
                it = mpool.tile([128, T1 * 8], dt.int16, tag="idx")
                nc.sync.dma_start(out=it[:], in_=idx1[b])
                dt_t = mpool.tile([128, T1], dt.float32, tag="dstl")
                nc.sync.dma_start(out=dt_t[:], in_=dstl1[b])

                # gathers chunked to <=512 idxs (SWDGE descriptor ring is 1024)
                CH = 4
                gb = gpool.tile([128, T1, G], dt.float32, tag="gb")
                for t0 in range(0, T_LO, CH):
                    n = min(CH, T_LO - t0)
                    nc.gpsimd.dma_gather(
                        gb[:, t0:t0 + n, :], HP[0:HLO, :],
                        it[:, t0 * 8:(t0 + n) * 8],
                        num_idxs=n * 128, num_idxs_reg=n * 128, elem_size=G)
                for t0 in range(0, T_HI, CH):
                    n = min(CH, T_HI - t0)
                    nc.gpsimd.dma_gather(
                        gb[:, T_LO + t0:T_LO + t0 + n, :], HP[HLO:VPAD, :],
                        it[:, (T_LO + t0) * 8:(T_LO + t0 + n) * 8],
                        num_idxs=n * 128, num_idxs_reg=n * 128, elem_size=G)

                ps = aggpsum.tile([128, G], dt.float32)
                for t0 in range(0, T1, CH):
                    n = min(CH, T1 - t0)
                    oh = ohpool.tile([128, CH, 128], dt.float32, tag="oh")
                    nc.vector.tensor_tensor(
                        out=oh[:, :n, :],
                        in0=dt_t[:, t0:t0 + n, None].to_broadcast([128, n, 128]),
                        in1=iota_t[:, None, :].to_broadcast([128, n, 128]),
                        op=OP.is_equal)
                    for t in range(n):
                        nc.tensor.matmul(out=ps[:], lhsT=oh[:, t, :],
                                         rhs=gb[:, t0 + t, :],
                                         start=(t0 + t == 0),
                                         stop=(t0 + t == T1 - 1))

                x2 = fpool.tile([128, G], dt.float32, tag="x2")
                nc.vector.tensor_scalar(
                    out=x2[:], in0=ps[:],
                    scalar1=invb[:, b:b + 1], scalar2=None, op0=OP.mult)
                nc.vector.tensor_add(out=x2[:], in0=x2[:], in1=b1_t[:])
                nc.scalar.activation(x2[:], x2[:], AF.Relu)

                # transpose x2 block -> X2T columns
                for j in range(2):
                    tp = trpsum.tile([128, 128], dt.float32)
                    nc.tensor.transpose(
                        out=tp[:], in_=x2[:, j * 128:(j + 1) * 128],
                        identity=ident[:])
                    x2tc = fpool.tile([128, 128], dt.float32, tag="x2t")
                    nc.vector.tensor_copy(out=x2tc[:], in_=tp[:])
                    nc.sync.dma_start(
                        out=X2T[j * 128:(j + 1) * 128, b * 128:(b + 1) * 128],
                        in_=x2tc[:])

            ctx_p2.__exit__(None, None, None)
            # ---------- phase 4: H2' = (x2 @ W2) * inv, local rows ----------
            ctx_p4 = nc.named_scope("p4_h2"); ctx_p4.__enter__()
            for r in range(NB1):
                lt = epool.tile([128, 2, 128], dt.float32, tag="x2tl")
                for j in range(2):
                    nc.sync.dma_start(
                        out=lt[:, j, :],
                        in_=X2T[j * 128:(j + 1) * 128, r * 128:(r + 1) * 128])
                ps = mm1psum.tile([128, G], dt.float32)
                for j in range(2):
                    nc.tensor.matmul(out=ps[:], lhsT=lt[:, j, :],
                                     rhs=w2_t[:, j, :],
                                     start=(j == 0), stop=(j == 1))
                h2 = hpool.tile([128, G], dt.float32, tag="h2")
                nc.vector.tensor_scalar(
                    out=h2[:], in0=ps[:],
                    scalar1=invb[:, r:r + 1], scalar2=None, op0=OP.mult)
                nc.sync.dma_start(out=H2P[r * 128:(r + 1) * 128, :], in_=h2[:])

            ctx_p4.__exit__(None, None, None)
            # ---------- phase 5: word partial aggregation ----------
            ctx_p5 = nc.named_scope("p5_word"); ctx_p5.__enter__()
            wpart = dram.tile([M * W_PAD, G], dt.float32)
            for b in range(NWB):
                it = mpool.tile([128, T_W * 8], dt.int16, tag="widx")
                nc.sync.dma_start(out=it[:], in_=widx[b])
                dt_t = mpool.tile([128, T_W], dt.float32, tag="wdstl")
                nc.sync.dma_start(out=dt_t[:], in_=wdstl[b])

                CH = 4
                gb = gpool.tile([128, T_W, G], dt.float32, tag="wgb")
                for t0 in range(0, T_W, CH):
                    n = min(CH, T_W - t0)
                    nc.gpsimd.dma_gather(
                        gb[:, t0:t0 + n, :], H2P[:], it[:, t0 * 8:(t0 + n) * 8],
                        num_idxs=n * 128, num_idxs_reg=n * 128, elem_size=G)

                ps = aggpsum.tile([128, G], dt.float32, tag="wps")
                for t0 in range(0, T_W, CH):
                    n = min(CH, T_W - t0)
                    oh = ohpool.tile([128, CH, 128], dt.float32, tag="woh")
                    nc.vector.tensor_tensor(
                        out=oh[:, :n, :],
                        in0=dt_t[:, t0:t0 + n, None].to_broadcast([128, n, 128]),
                        in1=iota_t[:, None, :].to_broadcast([128, n, 128]),
                        op=OP.is_equal)
                    for t in range(n):
                        nc.tensor.matmul(out=ps[:], lhsT=oh[:, t, :],
                                         rhs=gb[:, t0 + t, :],
                                         start=(t0 + t == 0),
                                         stop=(t0 + t == T_W - 1))
                pt = fpool.tile([128, G], dt.float32, tag="wpt")
                nc.vector.tensor_copy(out=pt[:], in_=ps[:])
                nc.sync.dma_start(out=wpart[b * 128:(b + 1) * 128, :], in_=pt[:])

            ctx_p5.__exit__(None, None, None)
            # ---------- phase 6: ReduceScatter partials ----------
            ctx_p6 = nc.named_scope("p6_rs"); ctx_p6.__enter__()
            wred = dram.tile([W_PAD, G], dt.float32)
            nc.gpsimd.collective_compute(
                "ReduceScatter", OP.add,
                replica_groups=[list(range(M))],
                ins=[wpart.opt()], outs=[wred.opt()])

            ctx_p6.__exit__(None, None, None)
            # ---------- phase 7: finalize owned word rows ----------
            ctx_p7 = nc.named_scope("p7_fin"); ctx_p7.__enter__()
            for t in range(W_PAD // 128):
                wt = fpool.tile([128, G], dt.float32, tag="wfin")
                nc.sync.dma_start(out=wt[:], in_=wred[t * 128:(t + 1) * 128, :])
                nc.vector.tensor_scalar(
                    out=wt[:], in0=wt[:],
                    scalar1=invw[:, t:t + 1], scalar2=None, op0=OP.mult)
                nc.vector.tensor_add(out=wt[:], in0=wt[:], in1=b2_t[:])
                nc.sync.dma_start(out=out_words[t * 128:(t + 1) * 128, :],
                                  in_=wt[:])

            ctx_p7.__exit__(None, None, None)
    nc.compile()
    return nc


def _install_ntff_hook():
    import sys, types
    import antenv
    from concourse import bass_utils
    if "antenv.axon_hooks" in sys.modules:
        return
    from trn_agent_boot.trn_boot import _ntff_profile_via_ctypes
    hooks = types.ModuleType("antenv.axon_hooks")
    _h = [None]
    hooks.set_axon_ntff_profile_hook = lambda h: _h.__setitem__(0, h)
    hooks.get_axon_ntff_profile_hook = lambda: _h[0]
    sys.modules["antenv.axon_hooks"] = hooks
    antenv.axon_hooks = hooks
    hooks.set_axon_ntff_profile_hook(
        _ntff_profile_via_ctypes("/opt/axon/libaxon_pjrt.so"))
    bass_utils.upload_artifacts = lambda tmpdir: f"local:{tmpdir}"


def kernel(**inputs):
    global last_exec_time_ns, last_results
    from concourse.bass_utils import run_bass_kernel_spmd

    cfg, in_maps, slot_map = _plan(
        inputs["emb"], inputs["W1"], inputs["b1"], inputs["W2"], inputs["b2"],
        inputs["edge_index"], inputs["word_ids"])

    if cfg not in _CACHE:
        _CACHE[cfg] = _build(cfg)
    nc = _CACHE[cfg]

    trace = os.environ.get("KERNEL_TRACE", "0") == "1"
    if trace:
        _install_ntff_hook()
    res = run_bass_kernel_spmd(nc, in_maps, core_ids=list(range(M)), trace=trace)
    last_exec_time_ns = res.exec_time_ns
    last_results = res

    W_PAD = cfg[2]
    word_ids = np.asarray(inputs["word_ids"], dtype=np.int64)
    all_rows = np.concatenate([res.results[c]["out_words"] for c in range(M)])
    out = all_rows[slot_map[word_ids.ravel()]].reshape(*word_ids.shape, G)
    return out.astype(np.float32)


# revision 10
# speedup vs baseline: 1.0376x; 1.0358x over previous
"""ConceptGNN (2-layer GCN + word gather) on 8 trn2 NeuronCores via Bass/Tile.

Strategy (hardcoded for V=50000, D=300, G=256, E=1.6e6, B=S=64, 8 cores):
  - Nodes dst-sharded: core c owns rows [c*6250, (c+1)*6250).
  - Layer 1: every core redundantly computes H1' = (emb @ W1) * rsqrt(deg)
    (cheaper than all-gathering a 50MB tensor), then aggregates only its own
    dst shard: edges bucketed by (dst block of 128, src half) on host,
    gathered edge-major with gpsimd.dma_gather (int16 indices -> two source
    halves), segment-summed by one-hot matmuls accumulating in PSUM.
    Self-loop terms are appended as explicit u->u edges so the device
    program is identical across cores (pure SPMD).
  - Layer 2: only rows needed by word_ids are produced. Each core computes
    H2' = (x2_shard @ W2) * rsqrt(deg) for its local rows, partial-aggregates
    word-destination edges whose src lives in its shard, ReduceScatter over
    the 8 cores (word slots are grouped by owner core), and finalizes its
    owned word rows. Host reassembles the (B,S,G) output by pure indexing.
"""

import os
import numpy as np

V, D, G = 50000, 300, 256
M = 8                      # cores
VS = V // M                # 6250 rows per shard
NB1 = (VS + 127) // 128    # 49 dst blocks per core
VPAD = 128 * ((V + 127) // 128 + 1)  # 50176, multiple of 128
HLO = 32768                # int16-addressable row limit for dma_gather
SUP = 4                    # row tiles per supertile in H' build

_CACHE = {}

# filled by the last kernel() call when KERNEL_TRACE=1
last_exec_time_ns = None
last_results = None


def _round_up(x, m):
    return (x + m - 1) // m * m


def _idx_tile16(arr_i16):
    """dma_gather index layout: idx k -> [k%16, k//16], replicated to 128 parts."""
    n = arr_i16.shape[0]
    t = arr_i16.reshape(n // 16, 16).T  # [16, n/16]
    return np.tile(t, (8, 1))           # [128, n/16]


def _col_tile(arr, T):
    """edge-major column layout: edge k -> [k%128, k//128]; arr len T*128."""
    return arr.reshape(T, 128).T        # [128, T]


def _plan(emb, W1, b1, W2, b2, edge_index, word_ids):
    """Host preprocessing: sharding, edge bucketing, padded index arrays."""
    src = np.asarray(edge_index[0], dtype=np.int64).astype(np.int32)
    dst = np.asarray(edge_index[1], dtype=np.int64).astype(np.int32)
    E = src.shape[0]

    deg = (1.0 + np.bincount(dst, minlength=V)).astype(np.float32)

    # append self edges (u -> u): covers the self-loop term of GCNConv
    src_all = np.concatenate([src, np.arange(V, dtype=np.int32)])
    dst_all = np.concatenate([dst, np.arange(V, dtype=np.int32)])

    # ---- layer-1 buckets: per (core, dst-block of 128, src-half) ----
    core_of = dst_all // VS
    dloc = dst_all % VS
    blk = dloc // 128
    half = (src_all >= HLO).astype(np.int32)

    per_core = []
    n_lo_max, n_hi_max = 1, 1
    for c in range(M):
        sel = np.where(core_of == c)[0]
        order = np.lexsort((half[sel], blk[sel]))
        sel = sel[order]
        s_c, b_c, h_c = src_all[sel], blk[sel], half[sel]
        dl_c = (dloc[sel] - b_c * 128).astype(np.float32)
        # boundaries per (block, half)
        buckets = []
        for b in range(NB1):
            ib = np.where(b_c == b)[0]
            ilo = ib[h_c[ib] == 0]
            ihi = ib[h_c[ib] == 1]
            buckets.append((s_c[ilo], dl_c[ilo], s_c[ihi] - HLO, dl_c[ihi]))
            n_lo_max = max(n_lo_max, len(ilo))
            n_hi_max = max(n_hi_max, len(ihi))
        per_core.append(buckets)

    T_LO = _round_up(n_lo_max, 128) // 128
    T_HI = _round_up(n_hi_max, 128) // 128
    T1 = T_LO + T_HI

    IDX1 = np.zeros((M, NB1, 128, T1 * 8), np.int16)
    DSTL1 = np.full((M, NB1, 128, T1), -1.0, np.float32)
    for c in range(M):
        for b in range(NB1):
            slo, dlo, shi, dhi = per_core[c][b]
            a = np.zeros(T_LO * 128, np.int16)
            a[: len(slo)] = slo.astype(np.int16)
            IDX1[c, b, :, : T_LO * 8] = _idx_tile16(a)
            d = np.full(T_LO * 128, -1.0, np.float32)
            d[: len(dlo)] = dlo
            DSTL1[c, b, :, :T_LO] = _col_tile(d, T_LO)
            a = np.zeros(T_HI * 128, np.int16)
            a[: len(shi)] = shi.astype(np.int16)
            IDX1[c, b, :, T_LO * 8:] = _idx_tile16(a)
            d = np.full(T_HI * 128, -1.0, np.float32)
            d[: len(dhi)] = dhi
            DSTL1[c, b, :, T_LO:] = _col_tile(d, T_HI)

    # ---- layer-2: word nodes, owner-grouped slots ----
    words = np.unique(np.asarray(word_ids, dtype=np.int64).astype(np.int32))
    owner = words // VS
    wlists = [words[owner == c] for c in range(M)]
    MW = max(max(len(w) for w in wlists), 1)
    W_PAD = _round_up(MW, 128)
    NWB = M * W_PAD // 128

    slot_map = np.full(V, -1, np.int64)
    for c in range(M):
        slot_map[wlists[c]] = c * W_PAD + np.arange(len(wlists[c]))

    wm = slot_map[dst] >= 0
    wsrc = np.concatenate([src[wm], words])
    wslot = np.concatenate([slot_map[dst[wm]], slot_map[words]]).astype(np.int32)

    wcore = wsrc // VS
    wblk = wslot // 128
    wloc = (wsrc % VS).astype(np.int32)
    wdl = (wslot % 128).astype(np.float32)

    n_w_max = 1
    wbuckets = []
    for c in range(M):
        sel = np.where(wcore == c)[0]
        order = np.argsort(wblk[sel], kind="stable")
        sel = sel[order]
        bl = []
        for b in range(NWB):
            ib = sel[wblk[sel] == b]
            bl.append((wloc[ib], wdl[ib]))
            n_w_max = max(n_w_max, len(ib))
        wbuckets.append(bl)
    T_W = _round_up(n_w_max, 128) // 128

    WIDX = np.zeros((M, NWB, 128, T_W * 8), np.int16)
    WDSTL = np.full((M, NWB, 128, T_W), -1.0, np.float32)
    for c in range(M):
        for b in range(NWB):
            ls, dl = wbuckets[c][b]
            a = np.zeros(T_W * 128, np.int16)
            a[: len(ls)] = ls.astype(np.int16)
            WIDX[c, b] = _idx_tile16(a)
            d = np.full(T_W * 128, -1.0, np.float32)
            d[: len(dl)] = dl
            WDSTL[c, b] = _col_tile(d, T_W)

    # ---- degree tensors ----
    degp = np.ones(VPAD, np.float32)
    degp[:V] = deg
    deg_glob = degp.reshape(VPAD // 128, 128).T.copy()  # [128, 392]

    deg_blk = np.ones((M, 128, NB1), np.float32)
    deg_w = np.ones((M, 128, W_PAD // 128), np.float32)
    for c in range(M):
        d = np.ones(NB1 * 128, np.float32)
        d[:VS] = deg[c * VS:(c + 1) * VS]
        deg_blk[c] = d.reshape(NB1, 128).T
        d = np.ones(W_PAD, np.float32)
        d[: len(wlists[c])] = deg[wlists[c]]
        deg_w[c] = d.reshape(W_PAD // 128, 128).T

    # ---- dense inputs ----
    embT = np.zeros((D, VPAD), np.float32)
    embT[:, :V] = np.asarray(emb, np.float32).T
    W1f = np.asarray(W1, np.float32)
    W2f = np.asarray(W2, np.float32)
    b1rep = np.broadcast_to(np.asarray(b1, np.float32), (128, G)).copy()
    b2rep = np.broadcast_to(np.asarray(b2, np.float32), (128, G)).copy()
    iota = np.broadcast_to(np.arange(128, dtype=np.float32), (128, 128)).copy()

    cfg = (T_LO, T_HI, W_PAD, NWB, T_W)
    in_maps = []
    for c in range(M):
        in_maps.append({
            "embT": embT, "W1": W1f, "W2": W2f,
            "b1rep": b1rep, "b2rep": b2rep, "iota": iota,
            "deg_glob": deg_glob, "deg_blk": deg_blk[c], "deg_w": deg_w[c],
            "idx1": IDX1[c], "dstl1": DSTL1[c],
            "widx": WIDX[c], "wdstl": WDSTL[c],
        })
    return cfg, in_maps, slot_map


def _build(cfg):
    from concourse import mybir, bacc
    import concourse.tile as tile

    T_LO, T_HI, W_PAD, NWB, T_W = cfg
    T1 = T_LO + T_HI
    dt = mybir.dt
    AF = mybir.ActivationFunctionType
    OP = mybir.AluOpType

    nc = bacc.Bacc("TRN2", target_bir_lowering=False, debug=False, num_devices=M)

    def din(name, shape, d=dt.float32):
        return nc.dram_tensor(name, shape, d, kind="ExternalInput").ap()

    embT = din("embT", [D, VPAD])
    W1 = din("W1", [D, G])
    W2 = din("W2", [G, G])
    b1rep = din("b1rep", [128, G])
    b2rep = din("b2rep", [128, G])
    iota_in = din("iota", [128, 128])
    deg_glob = din("deg_glob", [128, VPAD // 128])
    deg_blk = din("deg_blk", [128, NB1])
    deg_w = din("deg_w", [128, W_PAD // 128])
    idx1 = din("idx1", [NB1, 128, T1 * 8], dt.int16)
    dstl1 = din("dstl1", [NB1, 128, T1])
    widx = din("widx", [NWB, 128, T_W * 8], dt.int16)
    wdstl = din("wdstl", [NWB, 128, T_W])

    out_words = nc.dram_tensor("out_words", [W_PAD, G], dt.float32,
                               kind="ExternalOutput").ap()

    HP = nc.dram_tensor("HP", [VPAD, 2 * G], dt.bfloat16).ap()   # H1' hi|lo
    X2T = nc.dram_tensor("X2T", [G, NB1 * 128], dt.float32).ap()  # x2 transposed
    H2P = nc.dram_tensor("H2P", [NB1 * 128, 2 * G], dt.bfloat16).ap()  # H2' hi|lo

    NGT = VPAD // 128          # 392 row tiles
    NSUP = NGT // SUP          # 98 supertiles

    with tile.TileContext(nc) as tc:
        with tc.tile_pool(name="const", bufs=1) as cpool, \
             tc.tile_pool(name="emb", bufs=3) as epool, \
             tc.tile_pool(name="hp", bufs=3) as hpool, \
             tc.tile_pool(name="mm1", bufs=2, space="PSUM") as mm1psum, \
             tc.tile_pool(name="gath", bufs=2) as gpool, \
             tc.tile_pool(name="oh", bufs=2) as ohpool, \
             tc.tile_pool(name="meta", bufs=3) as mpool, \
             tc.tile_pool(name="agg", bufs=2, space="PSUM") as aggpsum, \
             tc.tile_pool(name="fin", bufs=3) as fpool, \
             tc.tile_pool(name="trp", bufs=2, space="PSUM") as trpsum, \
             tc.tile_pool(name="dram", bufs=1, space="DRAM") as dram:

            # ---------- constants ----------
            iota_t = cpool.tile([128, 128], dt.float32)
            nc.sync.dma_start(out=iota_t[:], in_=iota_in[:])
            ident = cpool.tile([128, 128], dt.float32)
            from concourse.masks import make_identity
            make_identity(nc, ident[:])
            KT = [(0, 128), (128, 128), (256, D - 256)]
            w1_t = cpool.tile([128, 3, G], dt.float32)
            for ki, (k0, kk) in enumerate(KT):
                nc.sync.dma_start(out=w1_t[:kk, ki, :], in_=W1[k0:k0 + kk, :])
            w2_t = cpool.tile([128, 2, G], dt.float32)
            for j in range(2):
                nc.sync.dma_start(out=w2_t[:, j, :], in_=W2[j * 128:(j + 1) * 128, :])
            b1_t = cpool.tile([128, G], dt.float32)
            nc.sync.dma_start(out=b1_t[:], in_=b1rep[:])
            b2_t = cpool.tile([128, G], dt.float32)
            nc.sync.dma_start(out=b2_t[:], in_=b2rep[:])

            invg = cpool.tile([128, VPAD // 128], dt.float32)
            nc.sync.dma_start(out=invg[:], in_=deg_glob[:])
            nc.scalar.activation(invg[:], invg[:], AF.Sqrt)
            nc.vector.reciprocal(invg[:], invg[:])
            invb = cpool.tile([128, NB1], dt.float32)
            nc.sync.dma_start(out=invb[:], in_=deg_blk[:])
            nc.scalar.activation(invb[:], invb[:], AF.Sqrt)
            nc.vector.reciprocal(invb[:], invb[:])
            invw = cpool.tile([128, W_PAD // 128], dt.float32)
            nc.sync.dma_start(out=invw[:], in_=deg_w[:])
            nc.scalar.activation(invw[:], invw[:], AF.Sqrt)
            nc.vector.reciprocal(invw[:], invw[:])

            # ---------- phase 1: H1' = (emb @ W1) * invg, replicated ----------
            ctx_p1 = nc.named_scope("p1_hprime"); ctx_p1.__enter__()
            for s in range(NSUP):
                et = epool.tile([128, 3, SUP * 128], dt.float32, tag="embt")
                for ki, (k0, kk) in enumerate(KT):
                    nc.sync.dma_start(
                        out=et[:kk, ki, :],
                        in_=embT[k0:k0 + kk, s * SUP * 128:(s + 1) * SUP * 128])
                hpt = hpool.tile([128, SUP, 2 * G], dt.bfloat16, tag="hp")
                hf = hpool.tile([128, G], dt.float32, tag="hf")
                for r in range(SUP):
                    ps = mm1psum.tile([128, G], dt.float32)
                    for ki, (k0, kk) in enumerate(KT):
                        nc.tensor.matmul(
                            out=ps[:],
                            lhsT=et[:kk, ki, r * 128:(r + 1) * 128],
                            rhs=w1_t[:kk, ki, :],
                            start=(ki == 0), stop=(ki == 2))
                    col = s * SUP + r
                    nc.vector.tensor_scalar(
                        out=hf[:], in0=ps[:],
                        scalar1=invg[:, col:col + 1], scalar2=None, op0=OP.mult)
                    nc.vector.tensor_copy(out=hpt[:, r, 0:G], in_=hf[:])
                    nc.vector.tensor_tensor(
                        out=hpt[:, r, G:2 * G], in0=hf[:], in1=hpt[:, r, 0:G],
                        op=OP.subtract)
                    nc.sync.dma_start(
                        out=HP[col * 128:(col + 1) * 128, :], in_=hpt[:, r, :])

            ctx_p1.__exit__(None, None, None)
            # ---------- phase 2: layer-1 aggregation over own dst shard ----------
            ctx_p2 = nc.named_scope("p2_agg"); ctx_p2.__enter__()
            for b in range(NB1):
                it = mpool.tile([128, T1 * 8], dt.int16, tag="idx")
                nc.sync.dma_start(out=it[:], in_=idx1[b])
                dt_t = mpool.tile([128, T1], dt.float32, tag="dstl")
                nc.sync.dma_start(out=dt_t[:], in_=dstl1[b])

                # gathers chunked to <=1024 idxs (SWDGE descriptor ring)
                CH = 8
                gb = gpool.tile([128, T1, 2 * G], dt.bfloat16, tag="gb")
                for t0 in range(0, T_LO, CH):
                    n = min(CH, T_LO - t0)
                    nc.gpsimd.dma_gather(
                        gb[:, t0:t0 + n, :], HP[0:HLO, :],
                        it[:, t0 * 8:(t0 + n) * 8],
                        num_idxs=n * 128, num_idxs_reg=n * 128, elem_size=2 * G)
                for t0 in range(0, T_HI, CH):
                    n = min(CH, T_HI - t0)
                    nc.gpsimd.dma_gather(
                        gb[:, T_LO + t0:T_LO + t0 + n, :], HP[HLO:VPAD, :],
                        it[:, (T_LO + t0) * 8:(T_LO + t0 + n) * 8],
                        num_idxs=n * 128, num_idxs_reg=n * 128, elem_size=2 * G)

                ps = aggpsum.tile([128, G], dt.float32)
                for t0 in range(0, T1, 4):
                    n = min(4, T1 - t0)
                    oh = ohpool.tile([128, 4, 128], dt.bfloat16, tag="oh")
                    nc.vector.tensor_tensor(
                        out=oh[:, :n, :],
                        in0=dt_t[:, t0:t0 + n, None].to_broadcast([128, n, 128]),
                        in1=iota_t[:, None, :].to_broadcast([128, n, 128]),
                        op=OP.is_equal)
                    for t in range(n):
                        for h in range(2):
                            nc.tensor.matmul(
                                out=ps[:], lhsT=oh[:, t, :],
                                rhs=gb[:, t0 + t, h * G:(h + 1) * G],
                                start=(t0 + t == 0 and h == 0),
                                stop=(t0 + t == T1 - 1 and h == 1))

                x2 = fpool.tile([128, G], dt.float32, tag="x2")
                nc.vector.tensor_scalar(
                    out=x2[:], in0=ps[:],
                    scalar1=invb[:, b:b + 1], scalar2=None, op0=OP.mult)
                nc.vector.tensor_add(out=x2[:], in0=x2[:], in1=b1_t[:])
                nc.scalar.activation(x2[:], x2[:], AF.Relu)

                # transpose x2 block -> X2T columns
                for j in range(2):
                    tp = trpsum.tile([128, 128], dt.float32)
                    nc.tensor.transpose(
                        out=tp[:], in_=x2[:, j * 128:(j + 1) * 128],
                        identity=ident[:])
                    x2tc = fpool.tile([128, 128], dt.float32, tag="x2t")
                    nc.vector.tensor_copy(out=x2tc[:], in_=tp[:])
                    nc.sync.dma_start(
                        out=X2T[j * 128:(j + 1) * 128, b * 128:(b + 1) * 128],
                        in_=x2tc[:])

            ctx_p2.__exit__(None, None, None)
            # ---------- phase 4: H2' = (x2 @ W2) * inv, local rows ----------
            ctx_p4 = nc.named_scope("p4_h2"); ctx_p4.__enter__()
            for r in range(NB1):
                lt = epool.tile([128, 2, 128], dt.float32, tag="x2tl")
                for j in range(2):
                    nc.sync.dma_start(
                        out=lt[:, j, :],
                        in_=X2T[j * 128:(j + 1) * 128, r * 128:(r + 1) * 128])
                ps = mm1psum.tile([128, G], dt.float32)
                for j in range(2):
                    nc.tensor.matmul(out=ps[:], lhsT=lt[:, j, :],
                                     rhs=w2_t[:, j, :],
                                     start=(j == 0), stop=(j == 1))
                h2f = hpool.tile([128, G], dt.float32, tag="h2f")
                nc.vector.tensor_scalar(
                    out=h2f[:], in0=ps[:],
                    scalar1=invb[:, r:r + 1], scalar2=None, op0=OP.mult)
                h2 = hpool.tile([128, 2 * G], dt.bfloat16, tag="h2")
                nc.vector.tensor_copy(out=h2[:, 0:G], in_=h2f[:])
                nc.vector.tensor_tensor(out=h2[:, G:2 * G], in0=h2f[:],
                                        in1=h2[:, 0:G], op=OP.subtract)
                nc.sync.dma_start(out=H2P[r * 128:(r + 1) * 128, :], in_=h2[:])

            ctx_p4.__exit__(None, None, None)
            # ---------- phase 5: word partial aggregation ----------
            ctx_p5 = nc.named_scope("p5_word"); ctx_p5.__enter__()
            wpart = dram.tile([M * W_PAD, G], dt.float32)
            for b in range(NWB):
                it = mpool.tile([128, T_W * 8], dt.int16, tag="widx")
                nc.sync.dma_start(out=it[:], in_=widx[b])
                dt_t = mpool.tile([128, T_W], dt.float32, tag="wdstl")
                nc.sync.dma_start(out=dt_t[:], in_=wdstl[b])

                CH = 8
                gb = gpool.tile([128, T_W, 2 * G], dt.bfloat16, tag="wgb")
                for t0 in range(0, T_W, CH):
                    n = min(CH, T_W - t0)
                    nc.gpsimd.dma_gather(
                        gb[:, t0:t0 + n, :], H2P[:], it[:, t0 * 8:(t0 + n) * 8],
                        num_idxs=n * 128, num_idxs_reg=n * 128, elem_size=2 * G)

                ps = aggpsum.tile([128, G], dt.float32, tag="wps")
                for t0 in range(0, T_W, 4):
                    n = min(4, T_W - t0)
                    oh = ohpool.tile([128, 4, 128], dt.bfloat16, tag="woh")
                    nc.vector.tensor_tensor(
                        out=oh[:, :n, :],
                        in0=dt_t[:, t0:t0 + n, None].to_broadcast([128, n, 128]),
                        in1=iota_t[:, None, :].to_broadcast([128, n, 128]),
                        op=OP.is_equal)
                    for t in range(n):
                        for h in range(2):
                            nc.tensor.matmul(
                                out=ps[:], lhsT=oh[:, t, :],
                                rhs=gb[:, t0 + t, h * G:(h + 1) * G],
                                start=(t0 + t == 0 and h == 0),
                                stop=(t0 + t == T_W - 1 and h == 1))
                pt = fpool.tile([128, G], dt.float32, tag="wpt")
                nc.vector.tensor_copy(out=pt[:], in_=ps[:])
                nc.sync.dma_start(out=wpart[b * 128:(b + 1) * 128, :], in_=pt[:])

            ctx_p5.__exit__(None, None, None)
            # ---------- phase 6: ReduceScatter partials ----------
            ctx_p6 = nc.named_scope("p6_rs"); ctx_p6.__enter__()
            wred = dram.tile([W_PAD, G], dt.float32)
            nc.gpsimd.collective_compute(
                "ReduceScatter", OP.add,
                replica_groups=[list(range(M))],
                ins=[wpart.opt()], outs=[wred.opt()])

            ctx_p6.__exit__(None, None, None)
            # ---------- phase 7: finalize owned word rows ----------
            ctx_p7 = nc.named_scope("p7_fin"); ctx_p7.__enter__()
            for t in range(W_PAD // 128):
                wt = fpool.tile([128, G], dt.float32, tag="wfin")
                nc.sync.dma_start(out=wt[:], in_=wred[t * 128:(t + 1) * 128, :])
                nc.vector.tensor_scalar(
                    out=wt[:], in0=wt[:],
                    scalar1=invw[:, t:t + 1], scalar2=None, op0=OP.mult)
                nc.vector.tensor_add(out=wt[:], in0=wt[:], in1=b2_t[:])
                nc.sync.dma_start(out=out_words[t * 128:(t + 1) * 128, :],
                                  in_=wt[:])

            ctx_p7.__exit__(None, None, None)
    nc.compile()
    return nc


def _install_ntff_hook():
    import sys, types
    import antenv
    from concourse import bass_utils
    if "antenv.axon_hooks" in sys.modules:
        return
    from trn_agent_boot.trn_boot import _ntff_profile_via_ctypes
    hooks = types.ModuleType("antenv.axon_hooks")
    _h = [None]
    hooks.set_axon_ntff_profile_hook = lambda h: _h.__setitem__(0, h)
    hooks.get_axon_ntff_profile_hook = lambda: _h[0]
    sys.modules["antenv.axon_hooks"] = hooks
    antenv.axon_hooks = hooks
    hooks.set_axon_ntff_profile_hook(
        _ntff_profile_via_ctypes("/opt/axon/libaxon_pjrt.so"))
    bass_utils.upload_artifacts = lambda tmpdir: f"local:{tmpdir}"


def kernel(**inputs):
    global last_exec_time_ns, last_results
    from concourse.bass_utils import run_bass_kernel_spmd

    cfg, in_maps, slot_map = _plan(
        inputs["emb"], inputs["W1"], inputs["b1"], inputs["W2"], inputs["b2"],
        inputs["edge_index"], inputs["word_ids"])

    if cfg not in _CACHE:
        _CACHE[cfg] = _build(cfg)
    nc = _CACHE[cfg]

    trace = os.environ.get("KERNEL_TRACE", "0") == "1"
    if trace:
        _install_ntff_hook()
    res = run_bass_kernel_spmd(nc, in_maps, core_ids=list(range(M)), trace=trace)
    last_exec_time_ns = res.exec_time_ns
    last_results = res

    W_PAD = cfg[2]
    word_ids = np.asarray(inputs["word_ids"], dtype=np.int64)
    all_rows = np.concatenate([res.results[c]["out_words"] for c in range(M)])
    out = all_rows[slot_map[word_ids.ravel()]].reshape(*word_ids.shape, G)
    return out.astype(np.float32)
